# revision 23
# baseline (speedup 1.0000x reference)
"""PinSageConv on 8 trn2 NeuronCores.

Full (unsharded) inputs in, full output out. Internally:
  - phase 1 sharded by src nodes (each core computes n = relu(h @ qw.T + qb)
    for its 1/8 of src rows, bf16), AllGather -> full n table in DRAM.
  - edges sharded by dst block (128 dst per block, blocks load-balanced
    across cores). Edges are bucketed (slot, src-chunk) where a chunk is a
    quarter of the n table (dma_gather indices are int16). Per 128-edge tile
    a dma_gather pulls the n rows (edges -> partitions), one DVE op builds
    the weighted one-hot O_w[e,d] = (iota[d]==dst_rel[e]) * w[e], and two PE
    matmuls accumulate agg = O_w.T @ n_g and ws = O_w.T @ ones in PSUM.
  - block tail: ws=max(ws,1); aggn=agg/ws; z=relu([h_dst|aggn] @ ww.T + wb)
    (bias via K=1 matmul); z/=||z||; DMA out. Host reassembles dst order.
"""

import os

import numpy as np
import ml_dtypes

import concourse.bass as bass
import concourse.bacc as bacc
import concourse.mybir as mybir
import concourse.tile as tile
from concourse.tile import add_dep_helper
from concourse.bass_utils import run_bass_kernel_spmd

BF16 = ml_dtypes.bfloat16
NCORES = 8
P = 128
NCHUNK = 4  # n-table split for int16 gather indices
GS = 7      # slots per gather group

LAST_RESULT = None  # BassKernelResults of the most recent run (for test.py)
LAST_SIM = None
LAST_META = None


def _balance_blocks(counts, ncores):
    """Assign dst blocks to (core, slot): big blocks first, serpentine."""
    nblk = len(counts)
    nslot = nblk // ncores
    order = np.argsort(-counts, kind="stable")
    assign = np.zeros((ncores, nslot), dtype=np.int64)
    for s in range(nslot):
        grp = order[s * ncores:(s + 1) * ncores]
        if s % 2:
            grp = grp[::-1]
        assign[:, s] = grp
    return assign


def _build_program(layout):
    """Build the SPMD Bass program (identical across cores; per-core data
    arrives via input tensors)."""
    L = layout
    nslot, nt, nph1 = L["nslot"], L["nt"], L["nph1"]
    nsh_rows, nfull_rows = L["nsh_rows"], L["nfull_rows"]
    chunk_rows = L["chunk_rows"]
    T2 = L["T2"]              # [nslot, NCHUNK] tiles per bucket
    colof = L["colof"]        # [nslot, NCHUNK] global col of bucket start
    calls = L["calls"]        # list of (g, c, cs, ce) gather calls
    gstart, tg = L["gstart"], L["tg"]
    ngroups = L["ngroups"]
    group_slots = L["group_slots"]

    nc = bacc.Bacc(num_devices=NCORES)
    f32, bf16 = mybir.dt.float32, mybir.dt.bfloat16
    AF = mybir.ActivationFunctionType
    OP = mybir.AluOpType

    NDSTP = nslot * P

    hT = nc.declare_dram_parameter("hT", [P, nsh_rows], bf16, isOutput=False)
    hdT = nc.declare_dram_parameter("hdT", [P, NDSTP], bf16, isOutput=False)
    qwT = nc.declare_dram_parameter("qwT", [P, P], bf16, isOutput=False)
    qb = nc.declare_dram_parameter("qb", [1, P], bf16, isOutput=False)
    wwT = nc.declare_dram_parameter("wwT", [P, 2 * P], bf16, isOutput=False)
    wb = nc.declare_dram_parameter("wb", [1, P], bf16, isOutput=False)
    iota_in = nc.declare_dram_parameter("iota", [P, P], bf16, isOutput=False)
    ident_in = nc.declare_dram_parameter("ident", [P, P], bf16, isOutput=False)
    idx_in = nc.declare_dram_parameter("idx", [P, nt * 8], mybir.dt.int16,
                                       isOutput=False)
    rel_in = nc.declare_dram_parameter("rel", [P, nt], f32, isOutput=False)
    wgt_in = nc.declare_dram_parameter("wgt", [P, nt], f32, isOutput=False)
    out = nc.declare_dram_parameter("out", [NDSTP, P], f32, isOutput=True)

    n_shard = nc.dram_tensor("n_shard", [nsh_rows, P], bf16)
    n_full = nc.dram_tensor("n_full", [nfull_rows, P], bf16,
                            addr_space="Shared")

    tgmax = int(max(tg)) if len(tg) else 1

    with tile.TileContext(nc) as tc:
        with (
            tc.tile_pool(name="const", bufs=1) as cpool,
            tc.tile_pool(name="ph1s", bufs=1) as ph1s,
            tc.tile_pool(name="ph1p", bufs=2, space="PSUM") as ph1p,
            tc.tile_pool(name="gpool", bufs=2) as gpool,
            tc.tile_pool(name="owp", bufs=4) as owp,
            tc.tile_pool(name="aggp", bufs=2, space="PSUM") as aggp,
            tc.tile_pool(name="wsp", bufs=2, space="PSUM") as wsp,
            tc.tile_pool(name="tzp", bufs=2, space="PSUM") as tzp,
            tc.tile_pool(name="tails", bufs=3) as tails,
        ):
            # ---- constants / per-core data into SBUF ----
            iota_t = cpool.tile([P, P], bf16)
            nc.sync.dma_start(iota_t[:], iota_in[:])
            ident_t = cpool.tile([P, P], bf16)
            nc.sync.dma_start(ident_t[:], ident_in[:])
            qwT_t = cpool.tile([P, P], bf16)
            nc.sync.dma_start(qwT_t[:], qwT[:])
            qb_t = cpool.tile([1, P], bf16)
            nc.sync.dma_start(qb_t[:], qb[:])
            wwT_t = cpool.tile([P, 2 * P], bf16)
            nc.sync.dma_start(wwT_t[:], wwT[:])
            wb_t = cpool.tile([1, P], bf16)
            nc.sync.dma_start(wb_t[:], wb[:])
            hdT_t = cpool.tile([P, NDSTP], bf16)
            nc.sync.dma_start(hdT_t[:], hdT[:])
            rel_t = cpool.tile([P, nt], f32)
            nc.sync.dma_start(rel_t[:], rel_in[:])
            wgt_t = cpool.tile([P, nt], f32)
            nc.sync.dma_start(wgt_t[:], wgt_in[:])
            ones_col = cpool.tile([P, 1], bf16)
            nc.vector.memset(ones_col[:], 1.0)
            eps_t = cpool.tile([P, 1], f32)
            nc.vector.memset(eps_t[:], 1e-30)
            ones_row = cpool.tile([1, P], bf16)
            nc.vector.memset(ones_row[:], 1.0)
            # idx loaded via gpsimd so the Pool engine clock covers it before
            # the gathers (fewer waits on the gather instructions).
            idxw_t = cpool.tile([P, nt * 8], mybir.dt.int16)
            nc.gpsimd.dma_start(idxw_t[:], idx_in[:])

            # ---- phase 1: n = relu(h @ qw.T + qb) for this core's shard ----
            hT_t = ph1s.tile([P, nph1 * P], bf16)
            nc.sync.dma_start(hT_t[:], hT[:, :nph1 * P])
            nst = ph1s.tile([P, nph1, P], bf16)
            for t in range(nph1):
                ps1 = ph1p.tile([P, P], f32, name=f"ps1_{t}", tag="ps1")
                nc.tensor.matmul(ps1[:], lhsT=hT_t[:, t * P:(t + 1) * P],
                                 rhs=qwT_t[:], start=True, stop=False)
                nc.tensor.matmul(ps1[:], lhsT=ones_row[:], rhs=qb_t[:],
                                 start=False, stop=True)
                nc.scalar.activation(nst[:, t, :], ps1[:], AF.Relu)
            nc.sync.dma_start(
                n_shard.rearrange("(t p) e -> p t e", p=P), nst[:])

            nc.gpsimd.collective_compute(
                "AllGather",
                mybir.AluOpType.bypass,
                replica_groups=[list(range(NCORES))],
                ins=[n_shard[:]],
                outs=[n_full[:]],
            )
            # absorb the collective wait on the Pool engine
            cc_probe = cpool.tile([1, P], bf16)
            nc.gpsimd.dma_start(cc_probe[:], n_full[0:1, :])

            # ---- edge loop ----
            prev_readers = {}
            for g in range(ngroups):
                if tg[g] == 0:
                    continue
                gb = gpool.tile([P, tgmax, P], bf16, name=f"gb_{g}", tag="gb")
                gis = []
                for (gg, c, cs, ce) in calls:
                    if gg != g or ce == cs:
                        continue
                    ni = (ce - cs) * P
                    gi = nc.gpsimd.dma_gather(
                        gb[:, cs - gstart[g]:ce - gstart[g], :],
                        n_full[c * chunk_rows:(c + 1) * chunk_rows, :],
                        idxw_t[:, cs * 8:ce * 8],
                        ni,
                        ni,
                        P,
                        single_packet=False,
                    )
                    for r in prev_readers.get(g - 2, []):
                        add_dep_helper(gi.ins, r, reason="gb slot WAR")
                    gis.append(gi.ins)
                readers = prev_readers[g] = []
                for s in group_slots[g]:
                    total_tiles = int(T2[s].sum())
                    if total_tiles == 0:
                        continue
                    pse = aggp.tile([P, P], f32, name=f"pse_{s}", tag="pse")
                    psw = wsp.tile([P, 1], f32, name=f"psw_{s}", tag="psw")
                    ti = 0
                    for c in range(NCHUNK):
                        for t in range(int(T2[s, c])):
                            gc = int(colof[s, c]) + t
                            lc = gc - gstart[g]
                            ow = owp.tile([P, P], bf16, name=f"ow_{gc}",
                                          tag="ow")
                            nc.vector.tensor_scalar(
                                ow[:], iota_t[:],
                                rel_t[:, gc:gc + 1], wgt_t[:, gc:gc + 1],
                                OP.is_equal, OP.mult)
                            mm = nc.tensor.matmul(
                                pse[:], lhsT=ow[:], rhs=gb[:, lc, :],
                                start=(ti == 0), stop=(ti == total_tiles - 1))
                            for gi_ in gis:
                                add_dep_helper(mm.ins, gi_, reason="gather RAW")
                            readers.append(mm.ins)
                            nc.tensor.matmul(
                                psw[:], lhsT=ow[:], rhs=ones_col[:],
                                start=(ti == 0), stop=(ti == total_tiles - 1))
                            ti += 1
                    # ---- block tail ----
                    if os.environ.get("PINSAGE_DEBUG_AGG", "0") == "1":
                        dbg = tails.tile([P, P], f32, name=f"dbg_{s}",
                                         tag="dbg")
                        nc.scalar.copy(dbg[:], pse[:])
                        nc.vector.tensor_copy(dbg[:, P - 1:P], psw[:])
                        nc.sync.dma_start(out[s * P:(s + 1) * P, :], dbg[:])
                        continue
                    stage = int(os.environ.get("PINSAGE_TAIL_STAGE", "4"))
                    wsc = tails.tile([P, 1], f32, name=f"wsc_{s}", tag="wsc")
                    nc.vector.tensor_scalar_max(wsc[:], psw[:], 1.0)
                    inv = tails.tile([P, 1], f32, name=f"inv_{s}", tag="inv")
                    nc.vector.reciprocal(inv[:], wsc[:])
                    aggn = tails.tile([P, P], bf16, name=f"aggn_{s}",
                                      tag="aggn")
                    nc.scalar.activation(aggn[:], pse[:], AF.Copy,
                                         scale=inv[:])
                    if stage == 1:
                        o1 = tails.tile([P, P], f32, name=f"o1_{s}", tag="o1")
                        nc.vector.tensor_copy(o1[:], aggn[:])
                        nc.sync.dma_start(out[s * P:(s + 1) * P, :], o1[:])
                        continue
                    ptr = tzp.tile([P, P], bf16, name=f"ptr_{s}", tag="tz")
                    nc.tensor.transpose(ptr[:], aggn[:], ident_t[:])
                    aggnT = tails.tile([P, P], bf16, name=f"aggnT_{s}",
                                       tag="aggnT")
                    nc.scalar.copy(aggnT[:], ptr[:])
                    if stage == 2:
                        o2 = tails.tile([P, P], f32, name=f"o2_{s}", tag="o2")
                        nc.vector.tensor_copy(o2[:], aggnT[:])
                        nc.sync.dma_start(out[s * P:(s + 1) * P, :], o2[:])
                        continue
                    pz = tzp.tile([P, P], f32, name=f"pz_{s}", tag="tz")
                    nc.tensor.matmul(pz[:], lhsT=hdT_t[:, s * P:(s + 1) * P],
                                     rhs=wwT_t[:, 0:P], start=True, stop=False)
                    nc.tensor.matmul(pz[:], lhsT=aggnT[:],
                                     rhs=wwT_t[:, P:2 * P],
                                     start=False, stop=False)
                    nc.tensor.matmul(pz[:], lhsT=ones_row[:], rhs=wb_t[:],
                                     start=False, stop=True)
                    z = tails.tile([P, P], f32, name=f"z_{s}", tag="z")
                    nc.scalar.activation(z[:], pz[:], AF.Relu)
                    if stage == 3:
                        nc.sync.dma_start(out[s * P:(s + 1) * P, :], z[:])
                        continue
                    sq = tails.tile([P, P], f32, name=f"sq_{s}", tag="sq")
                    nsq = tails.tile([P, 1], f32, name=f"nsq_{s}", tag="nsq")
                    nc.scalar.activation(sq[:], z[:], AF.Square,
                                         accum_out=nsq[:])
                    if stage == 31:
                        nc.sync.dma_start(out[s * P:(s + 1) * P, :], sq[:])
                        continue
                    nrm = tails.tile([P, 1], f32, name=f"nrm_{s}", tag="nrm")
                    nc.scalar.activation(nrm[:], nsq[:], AF.Sqrt,
                                         bias=eps_t[:], scale=1.0)
                    invn = tails.tile([P, 1], f32, name=f"invn_{s}",
                                      tag="invn")
                    nc.vector.reciprocal(invn[:], nrm[:])
                    ot = tails.tile([P, P], f32, name=f"ot_{s}", tag="ot")
                    if stage == 32:
                        nc.scalar.activation(ot[:], z[:], AF.Copy,
                                             scale=1.0)
                        nc.sync.dma_start(out[s * P:(s + 1) * P, :], ot[:])
                        continue
                    nc.scalar.activation(ot[:], z[:], AF.Copy, scale=invn[:])
                    nc.sync.dma_start(out[s * P:(s + 1) * P, :], ot[:])

    nc.compile()
    return nc


def prepare(h_src, weights, src_idx, dst_idx, num_dst, q_w, q_b, w_w, w_b):
    """Host-side sharding + program build. Returns (nc, in_maps, meta)."""
    h_src = np.asarray(h_src, dtype=np.float32)
    weights = np.asarray(weights, dtype=np.float32)
    src_idx = np.asarray(src_idx, dtype=np.int64)
    dst_idx = np.asarray(dst_idx, dtype=np.int64)
    num_dst = int(num_dst)
    q_w = np.asarray(q_w, dtype=np.float32)
    q_b = np.asarray(q_b, dtype=np.float32)
    w_w = np.asarray(w_w, dtype=np.float32)
    w_b = np.asarray(w_b, dtype=np.float32)

    n_src, E = h_src.shape
    assert E == P

    # ---------- host-side sharding / layout ----------
    shard = (n_src + NCORES - 1) // NCORES
    nph1 = (shard + P - 1) // P
    nsh_rows = nph1 * P
    nfull_rows = nsh_rows * NCORES
    assert nfull_rows % NCHUNK == 0
    chunk_rows = nfull_rows // NCHUNK
    assert chunk_rows <= 32767, chunk_rows

    nblk_real = (num_dst + P - 1) // P
    nslot = (nblk_real + NCORES - 1) // NCORES
    nblk = nslot * NCORES
    blk = dst_idx // P
    bcounts = np.bincount(blk, minlength=nblk)
    assign = _balance_blocks(bcounts, NCORES)

    core_of_blk = np.zeros(nblk, dtype=np.int64)
    slot_of_blk = np.zeros(nblk, dtype=np.int64)
    for c in range(NCORES):
        for s in range(nslot):
            core_of_blk[assign[c, s]] = c
            slot_of_blk[assign[c, s]] = s

    # n-table row of each src node; chunk of each edge
    n_row_of_src = (src_idx // shard) * nsh_rows + (src_idx % shard)
    echunk_of_src = n_row_of_src // chunk_rows
    eloc_of_src = n_row_of_src - echunk_of_src * chunk_rows

    ecore = core_of_blk[blk]
    eslot = slot_of_blk[blk]
    echunk = echunk_of_src
    erel = (dst_idx - blk * P).astype(np.float32)

    # bucket counts per (core, slot, chunk) -> shared T2 = max over cores
    bid = (ecore * nslot + eslot) * NCHUNK + echunk
    bc = np.bincount(bid, minlength=NCORES * nslot * NCHUNK)
    bc = bc.reshape(NCORES, nslot, NCHUNK)
    T2 = ((bc.max(axis=0) + P - 1) // P).astype(np.int64)  # [nslot, NCHUNK]

    # global tile-column layout: groups of GS slots; within a group:
    # chunk-major, then slot order, then tiles
    ngroups = (nslot + GS - 1) // GS
    group_slots = [list(range(g * GS, min((g + 1) * GS, nslot)))
                   for g in range(ngroups)]
    colof = np.zeros((nslot, NCHUNK), dtype=np.int64)
    gstart = np.zeros(ngroups + 1, dtype=np.int64)
    calls = []  # (g, chunk, col_start, col_end)
    col = 0
    for g in range(ngroups):
        gstart[g] = col
        for c in range(NCHUNK):
            cs = col
            for s in group_slots[g]:
                colof[s, c] = col
                col += int(T2[s, c])
            calls.append((g, c, cs, col))
    gstart[ngroups] = col
    nt = int(col)
    tg = np.array([gstart[g + 1] - gstart[g] for g in range(ngroups)],
                  dtype=np.int64)

    # ---------- per-core packed arrays ----------
    idx_arrs, rel_arrs, wgt_arrs = [], [], []
    skey = bid  # (core, slot, chunk) combined
    sidx = np.argsort(skey, kind="stable")
    skey_s = skey[sidx]
    bstride = nslot * NCHUNK
    for c in range(NCORES):
        sel = sidx[(skey_s >= c * bstride) & (skey_s < (c + 1) * bstride)]
        sk = skey_s[(skey_s >= c * bstride) & (skey_s < (c + 1) * bstride)] \
            - c * bstride  # slot*NCHUNK + chunk
        cnt = np.bincount(sk, minlength=bstride)
        cum = np.concatenate([[0], np.cumsum(cnt)])[:-1]
        j = np.arange(len(sel)) - cum[sk]  # rank within bucket
        s_of = sk // NCHUNK
        c_of = sk % NCHUNK
        gcol = colof[s_of, c_of] + j // P
        p = j % P
        ia = np.zeros((P, nt), dtype=np.int16)
        ra = np.zeros((P, nt), dtype=np.float32)
        wa = np.zeros((P, nt), dtype=np.float32)
        ia[p, gcol] = eloc_of_src[sel].astype(np.int16)
        ra[p, gcol] = erel[sel]
        wa[p, gcol] = weights[sel]
        # wrapped idx layout: per tile col t, its 128 idxs i=(t*128+p) live at
        # wrapped[(i%16), i//16] -> within-tile: wrapped cols [t*8,(t+1)*8)
        flat = ia.T.reshape(-1)  # i = t*128 + p order
        wrapped = flat.reshape(-1, 16).T  # [16, nt*8]
        iw = np.tile(wrapped, (8, 1)).astype(np.int16)  # [128, nt*8]
        idx_arrs.append(np.ascontiguousarray(iw))
        rel_arrs.append(ra)
        wgt_arrs.append(wa)

    # phase-1 input: per-core transposed src slice
    h_bf = h_src.astype(BF16)
    hT_arrs = []
    for c in range(NCORES):
        hTa = np.zeros((P, nsh_rows), dtype=BF16)
        lo, hi = c * shard, min((c + 1) * shard, n_src)
        hTa[:, :hi - lo] = h_bf[lo:hi].T
        hT_arrs.append(np.ascontiguousarray(hTa))

    # h_dst slices in (core, slot) order
    hdT_arrs = []
    NDSTP = nslot * P
    for c in range(NCORES):
        hd = np.zeros((P, NDSTP), dtype=BF16)
        for s in range(nslot):
            b = assign[c, s]
            lo = b * P
            hi = min(lo + P, num_dst)
            if hi > lo:
                hd[:, s * P:s * P + (hi - lo)] = h_bf[lo:hi].T
        hdT_arrs.append(hd)

    iota = np.tile(np.arange(P, dtype=np.float32), (P, 1)).astype(BF16)
    ident = np.eye(P, dtype=np.float32).astype(BF16)
    qwT = np.ascontiguousarray(q_w.T.astype(BF16))
    qb_a = q_b.reshape(1, P).astype(BF16)
    wwT = np.ascontiguousarray(
        w_w.T.astype(BF16).reshape(2, P, P).transpose(1, 0, 2).reshape(P, 2 * P))
    wb_a = w_b.reshape(1, P).astype(BF16)

    in_maps = []
    for c in range(NCORES):
        in_maps.append({
            "hT": hT_arrs[c],
            "hdT": hdT_arrs[c],
            "qwT": qwT,
            "qb": qb_a,
            "wwT": wwT,
            "wb": wb_a,
            "iota": iota,
            "ident": ident,
            "idx": idx_arrs[c],
            "rel": rel_arrs[c],
            "wgt": wgt_arrs[c],
        })

    layout = dict(nslot=nslot, nt=nt, nph1=nph1, nsh_rows=nsh_rows,
                  nfull_rows=nfull_rows, chunk_rows=chunk_rows, T2=T2,
                  colof=colof, calls=calls, gstart=gstart, tg=tg,
                  ngroups=ngroups, group_slots=group_slots)
    nc = _build_program(layout)
    meta = dict(assign=assign, nslot=nslot, num_dst=num_dst, layout=layout,
                T2=T2, colof=colof, idx_arrs=idx_arrs, rel_arrs=rel_arrs,
                wgt_arrs=wgt_arrs, nsh_rows=nsh_rows)
    return nc, in_maps, meta


def unshard(results, meta):
    num_dst, nslot = meta["num_dst"], meta["nslot"]
    assign = meta["assign"]
    out_full = np.zeros((num_dst, P), dtype=np.float32)
    for c in range(NCORES):
        o = results[c]["out"]
        for s in range(nslot):
            b = assign[c, s]
            lo = b * P
            hi = min(lo + P, num_dst)
            if hi > lo:
                out_full[lo:hi] = o[s * P:s * P + (hi - lo)]
    return out_full


def kernel(h_src, weights, src_idx, dst_idx, num_dst, q_w, q_b, w_w, w_b):
    global LAST_RESULT, LAST_SIM, LAST_META
    nc, in_maps, meta = prepare(h_src, weights, src_idx, dst_idx, num_dst,
                                q_w, q_b, w_w, w_b)
    if os.environ.get("PINSAGE_SIM", "0") == "1":
        from concourse.bass_interp import MultiCoreSim
        sim = MultiCoreSim(nc, num_cores=NCORES)
        for c in range(NCORES):
            for k, v in in_maps[c].items():
                sim.cores[c].tensor(k)[:] = v
        sim.simulate()
        results = [{"out": np.array(sim.cores[c].tensor("out"))}
                   for c in range(NCORES)]
        res = None
        LAST_SIM = sim
        LAST_META = meta
    else:
        trace = os.environ.get("PINSAGE_TRACE", "0") == "1"
        res = run_bass_kernel_spmd(nc, in_maps, core_ids=list(range(NCORES)),
                                   trace=trace)
        results = res.results
    LAST_RESULT = res
    return unshard(results, meta)
